# revision 1
# baseline (speedup 1.0000x reference)
"""Trainium2 Bass kernel for GQA attention forward (B=2, S=2048, D=2048,
16 q-heads / 4 kv-heads, head_dim=128, RoPE, causal).

Sharding: 8 cores = 2 (batch) x 4 (kv-head groups). Each core computes its
batch's attention for one kv-head group (4 q-heads + 1 kv head) and a
row-parallel partial of the output projection; the host sums the 4 partials
per batch.

Matmul operands are bf16 (1 cycle/row on PE) with fp32 PSUM accumulation;
the softmax denominator path runs in fp32/f32r to avoid bf16 rounding of
the normalization.
"""

import sys

if "/opt/trn_rl_repo" not in sys.path:
    sys.path.insert(0, "/opt/trn_rl_repo")

import numpy as np
import ml_dtypes

import concourse.bass as bass
import concourse.tile as tile
from concourse import mybir

F32 = mybir.dt.float32
F32R = mybir.dt.float32r
BF16 = mybir.dt.bfloat16

# Full-problem constants (per reference).
B, S, DIM = 2, 2048, 2048
N_HEADS, N_KV_HEADS, HEAD_DIM = 16, 4, 128
N_GROUPS = N_KV_HEADS          # tensor-parallel groups
HQ = N_HEADS // N_KV_HEADS     # q heads per group
NEG = -1e30


def build_attention_core(nc, S=S, D=DIM, HQ=HQ, HD=HEAD_DIM, CHUNK=512):
    """Emit the per-core attention program into `nc` (Tile framework).

    Inputs (ExternalInput dram tensors):
      x      [S, D]  bf16   activations for this core's batch
      wqT    [D, HQ*HD] bf16  q projection, transposed, RoPE-permuted rows
      wkvT   [D, 2*HD] bf16   [wk^T | wv^T] (wk RoPE-permuted)
      woT    [HQ*HD, D] bf16  output projection slice, transposed
      t1,t2  [S, HD] f32      RoPE tables (permuted-half layout)
      masks  [CHUNK//128, 128, CHUNK] f32 additive causal masks
      ident  [128, 128] bf16  identity for PE transposes
      ones_col [128,1] bf16 / ones_row [1,128] f32r
    Output:
      out_partial [S, D] f32
    """
    n_st = S // 128        # s tiles
    n_dt = D // 128        # d tiles
    n_ch = S // CHUNK      # q chunks
    kpc = CHUNK // 128     # k-tiles per chunk
    n_dc = D // CHUNK      # d chunks (phase C)
    IQ = HQ * HD

    x_d = nc.dram_tensor("xT", [128, D // 128, S], BF16, kind="ExternalInput")
    wqT_d = nc.dram_tensor("wqT", [128, D // 128, IQ], BF16, kind="ExternalInput")
    wkvT_d = nc.dram_tensor("wkvT", [128, D // 128, 2 * HD], BF16, kind="ExternalInput")
    woT_d = nc.dram_tensor("woT", [128, IQ // 128, D], BF16, kind="ExternalInput")
    t1_d = nc.dram_tensor("t1", [128, S // 128, HD], F32, kind="ExternalInput")
    t2_d = nc.dram_tensor("t2", [128, S // 128, HD], F32, kind="ExternalInput")
    masks_d = nc.dram_tensor("masks", [128, kpc, CHUNK], F32, kind="ExternalInput")
    ident_d = nc.dram_tensor("ident", [128, 128], BF16, kind="ExternalInput")
    onesc_d = nc.dram_tensor("ones_col", [128, 1], BF16, kind="ExternalInput")
    onesr_d = nc.dram_tensor("ones_row", [1, 128], F32R, kind="ExternalInput")
    out_d = nc.dram_tensor("out_partial", [S, D], F32, kind="ExternalOutput")

    scale = float(HD) ** -0.5

    with tile.TileContext(nc) as tc:
        with (
            # tensors persistent across phases
            tc.tile_pool(name="persist", bufs=1) as persist,
            tc.tile_pool(name="constB", bufs=1) as constB,
        ):
            qT_sb = persist.tile([128, HQ, S], BF16)    # [e, h, s]
            kT_sb = persist.tile([128, S], BF16)        # [e, s]
            v_sb = persist.tile([128, n_st, HD], BF16)  # [s_in_tile, s_tile, e]
            oT_sb = persist.tile([128, HQ, S], BF16)    # [e, h, s]

            # ---------------- Phase A: projections + RoPE -------------------
            with (
                tc.tile_pool(name="weightsA", bufs=1) as weightsA,
                tc.tile_pool(name="xt", bufs=1) as xt_pool,
                tc.tile_pool(name="rope", bufs=4) as rope_pool,
                tc.tile_pool(name="ps_t", bufs=3, space="PSUM") as pst_pool,
                tc.tile_pool(name="ps_q", bufs=2, space="PSUM") as psq_pool,
                tc.tile_pool(name="ps_kv", bufs=2, space="PSUM") as pskv_pool,
            ):
                gq = n_dt // 4
                wq_g = []
                wkv_g = []
                for g in range(4):
                    wqg = weightsA.tile([128, gq, IQ], BF16, tag=f"wq{g}",
                                        name=f"wq{g}")
                    nc.scalar.dma_start(
                        out=wqg, in_=wqT_d[:, g * gq:(g + 1) * gq, :]
                    )
                    wq_g.append(wqg)
                    wkvg = weightsA.tile([128, gq, 2 * HD], BF16, tag=f"wkv{g}",
                                         name=f"wkv{g}")
                    nc.scalar.dma_start(
                        out=wkvg, in_=wkvT_d[:, g * gq:(g + 1) * gq, :]
                    )
                    wkv_g.append(wkvg)
                ident = weightsA.tile([128, 128], BF16)
                nc.scalar.dma_start(out=ident, in_=ident_d[:])
                t1_sb = weightsA.tile([128, n_st, HD], F32)
                nc.scalar.dma_start(
                    out=t1_sb, in_=t1_d[:]
                )
                t2_sb = weightsA.tile([128, n_st, HD], F32)
                nc.scalar.dma_start(
                    out=t2_sb, in_=t2_d[:]
                )

                # pre-transposed activations: 2 d-tiles per DMA, sync queue
                xpair = []
                for g2 in range(n_dt // 2):
                    xt_t = xt_pool.tile([128, 2, S], BF16, tag=f"xt{g2}")
                    nc.sync.dma_start(out=xt_t, in_=x_d[:, g2 * 2:g2 * 2 + 2, :])
                    xpair.append(xt_t)
                xT = [xpair[dt_ // 2][:, dt_ % 2, :] for dt_ in range(n_dt)]

                # phase-B constants loaded early so the first diagonal
                # mask-add never waits
                masks_sb = constB.tile([128, kpc, CHUNK], F32)
                nc.sync.dma_start(out=masks_sb, in_=masks_d[:])
                ones_col = constB.tile([128, 1], BF16)
                nc.sync.dma_start(out=ones_col, in_=onesc_d[:])
                ones_row = constB.tile([1, 128], F32R)
                nc.sync.dma_start(out=ones_row, in_=onesr_d[:])

                rope_pending = None

                def emit_transposes(rp):
                    q_rot_, k_rot_, sl_ = rp
                    for h in range(HQ):
                        ps_tq = pst_pool.tile([128, 128], BF16, tag="ps_t")
                        nc.tensor.transpose(
                            ps_tq, q_rot_[:, h * HD:(h + 1) * HD], ident
                        )
                        nc.vector.tensor_copy(qT_sb[:, h, sl_], ps_tq)
                    ps_tk = pst_pool.tile([128, 128], BF16, tag="ps_t")
                    nc.tensor.transpose(ps_tk, k_rot_, ident)
                    nc.vector.tensor_copy(kT_sb[:, sl_], ps_tk)

                for st in range(n_st):
                    ps_q = psq_pool.tile([128, IQ], F32)
                    ps_kv = pskv_pool.tile([128, 2 * HD], F32)
                    st_sl = slice(st * 128, (st + 1) * 128)
                    for dt_ in range(n_dt):
                        nc.tensor.matmul(
                            ps_kv, xT[dt_][:, st_sl], wkv_g[dt_ // gq][:, dt_ % gq, :],
                            start=(dt_ == 0), stop=(dt_ == n_dt - 1),
                        )
                        nc.tensor.matmul(
                            ps_q, xT[dt_][:, st_sl], wq_g[dt_ // gq][:, dt_ % gq, :],
                            start=(dt_ == 0), stop=(dt_ == n_dt - 1),
                        )
                    # previous s-tile's PE transposes: emitted here so PE
                    # never waits on the DVE RoPE chain
                    if rope_pending is not None:
                        emit_transposes(rope_pending)
                        rope_pending = None

                    # RoPE on all q heads at once (tables broadcast
                    # across heads via zero-stride AP)
                    t1s = t1_sb[:, st, :]
                    t2s = t2_sb[:, st, :]
                    t1b = bass.AP(tensor=t1s.tensor, offset=t1s.offset,
                                  ap=[t1s.ap[0], [0, HQ], t1s.ap[1]])
                    t2b = bass.AP(tensor=t2s.tensor, offset=t2s.offset,
                                  ap=[t2s.ap[0], [0, HQ], t2s.ap[1]])
                    ps_qv = ps_q.rearrange("p (h e) -> p h e", h=HQ)
                    t1m = rope_pool.tile([128, HQ, HD], F32, tag="t1m")
                    nc.vector.tensor_mul(t1m, ps_qv, t1b)
                    t2m = rope_pool.tile([128, HQ, HD], F32, tag="t2m")
                    nc.vector.tensor_mul(
                        t2m[:, :, 0:64], ps_qv[:, :, 64:128], t2b[:, :, 0:64]
                    )
                    nc.vector.tensor_mul(
                        t2m[:, :, 64:128], ps_qv[:, :, 0:64], t2b[:, :, 64:128]
                    )
                    q_rot = rope_pool.tile([128, HQ * HD], BF16, tag="qrot")
                    nc.vector.tensor_add(
                        q_rot.rearrange("p (h e) -> p h e", h=HQ), t1m, t2m
                    )
                    # RoPE on k
                    t1mk = rope_pool.tile([128, HD], F32, tag="t1mk")
                    nc.vector.tensor_mul(t1mk, ps_kv[:, 0:HD], t1_sb[:, st, :])
                    t2mk = rope_pool.tile([128, HD], F32, tag="t2mk")
                    nc.vector.tensor_mul(
                        t2mk[:, 0:64], ps_kv[:, 64:128], t2_sb[:, st, 0:64]
                    )
                    nc.vector.tensor_mul(
                        t2mk[:, 64:128], ps_kv[:, 0:64], t2_sb[:, st, 64:128]
                    )
                    k_rot = rope_pool.tile([128, HD], BF16, tag="krot")
                    nc.vector.tensor_add(k_rot, t1mk, t2mk)
                    rope_pending = (q_rot, k_rot, st_sl)

                    # v: straight copy (natural [s, e] layout), cast to bf16
                    nc.scalar.copy(v_sb[:, st, :], ps_kv[:, HD:2 * HD])
                emit_transposes(rope_pending)

            # ---------------- Phases B+C shared: woT ------------------------
            with tc.tile_pool(name="weightsC", bufs=1) as weightsC:
                woT_sb = weightsC.tile([128, IQ // 128, D], BF16)
                nc.sync.dma_start(
                    out=woT_sb, in_=woT_d[:]
                )

                # ---------------- Phase B: attention ------------------------
                with (
                    tc.tile_pool(name="expt", bufs=4) as expt_pool,
                    tc.tile_pool(name="maskbuf", bufs=3) as mask_pool,
                    tc.tile_pool(name="sums", bufs=2) as sums_pool,
                    tc.tile_pool(name="recip", bufs=2) as rec_pool,
                    tc.tile_pool(name="ps_s", bufs=3, space="PSUM") as pss_pool,
                    tc.tile_pool(name="ps_o", bufs=2, space="PSUM") as pso_pool,
                    tc.tile_pool(name="ps_sum", bufs=2, space="PSUM") as pssum_pool,
                    tc.tile_pool(name="ps_b", bufs=1, space="PSUM") as psb_pool,
                ):
                    norm_pending = [None]

                    def emit_norm():
                        ps_o_, ps_sum_, h_, c_ = norm_pending[0]
                        norm_pending[0] = None
                        sums_sb = sums_pool.tile([1, CHUNK], F32R, tag="sums")
                        with nc.allow_low_precision(reason="f32r denom"):
                            nc.vector.tensor_copy(sums_sb, ps_sum_)
                        ps_b = psb_pool.tile([128, CHUNK], F32)
                        nc.tensor.matmul(
                            ps_b, ones_row, sums_sb, start=True, stop=True,
                        )
                        recip = rec_pool.tile([128, CHUNK], F32)
                        nc.vector.reciprocal_approx_fast(recip, ps_b)
                        nc.vector.tensor_mul(
                            oT_sb[:, h_, c_ * CHUNK:(c_ + 1) * CHUNK],
                            ps_o_, recip,
                        )

                    for h in range(HQ):
                        for c in range(n_ch):
                            ps_o = pso_pool.tile([128, CHUNK], F32)
                            ps_sum = pssum_pool.tile([1, CHUNK], F32)
                            n_kj = (c + 1) * kpc
                            c_sl = slice(c * CHUNK, (c + 1) * CHUNK)
                            pending = []

                            def flush_one():
                                pe, pj, poff = pending.pop(0)
                                nc.tensor.matmul(
                                    ps_o[:, poff:], v_sb[:, pj, :], pe,
                                    start=(pj == 0), stop=(pj == n_kj - 1),
                                )
                                nc.tensor.matmul(
                                    ps_sum[:, poff:], ones_col, pe,
                                    start=(pj == 0), stop=(pj == n_kj - 1),
                                )

                            for kj in range(n_kj):
                                # columns left of the diagonal block are fully
                                # masked: skip them (q >= kj*128 only)
                                off = max(0, (kj - c * kpc)) * 128
                                w = CHUNK - off
                                ps_s = pss_pool.tile([128, CHUNK], F32, tag="ps_s")
                                nc.tensor.matmul(
                                    ps_s[:, 0:w],
                                    kT_sb[:, kj * 128:(kj + 1) * 128],
                                    qT_sb[:, h, c * CHUNK + off:(c + 1) * CHUNK],
                                    start=True, stop=True,
                                )
                                if kj >= c * kpc:  # diagonal chunk: causal mask
                                    msk = mask_pool.tile([128, CHUNK], F32, tag="msk")
                                    nc.vector.tensor_add(
                                        msk[:, 0:w], ps_s[:, 0:w],
                                        masks_sb[:, kj % kpc, off:],
                                    )
                                    exp_in = msk
                                else:
                                    exp_in = ps_s
                                expT = expt_pool.tile([128, CHUNK], BF16, tag="expT")
                                nc.scalar.activation(
                                    expT[:, 0:w], exp_in[:, 0:w],
                                    mybir.ActivationFunctionType.Exp,
                                    scale=scale,
                                )
                                pending.append((expT[:, 0:w], kj, off))
                                if kj == 1 and norm_pending[0] is not None:
                                    emit_norm()
                                if len(pending) > 2:
                                    flush_one()
                            while pending:
                                flush_one()
                            norm_pending[0] = (ps_o, ps_sum, h, c)

                    emit_norm()

                # ---------------- Phase C: output projection ----------------
                with (
                    tc.tile_pool(name="outsb", bufs=2) as outsb_pool,
                    tc.tile_pool(name="ps_d", bufs=4, space="PSUM") as psd_pool,
                ):
                    for st in range(n_st):
                        out_sb = outsb_pool.tile([128, D], F32)
                        for dc in range(n_dc):
                            ps_d = psd_pool.tile([128, CHUNK], F32)
                            for it in range(HQ):
                                nc.tensor.matmul(
                                    ps_d,
                                    oT_sb[:, it, st * 128:(st + 1) * 128],
                                    woT_sb[:, it, dc * CHUNK:(dc + 1) * CHUNK],
                                    start=(it == 0), stop=(it == HQ - 1),
                                )
                            nc.scalar.copy(
                                out_sb[:, dc * CHUNK:(dc + 1) * CHUNK], ps_d
                            )
                        nc.sync.dma_start(
                            out=out_d[st * 128:(st + 1) * 128, :], in_=out_sb
                        )

    return nc


# ---------------------------------------------------------------------------
# Host-side prep


_ROPE_PERM = np.concatenate([np.arange(0, HEAD_DIM, 2), np.arange(1, HEAD_DIM, 2)])


def _prep_tables(freq_cis, S_=S, HD_=HEAD_DIM):
    """RoPE tables in permuted-half layout: rot = q*t1 + swap(q)*t2."""
    fc = np.asarray(freq_cis, dtype=np.float32)
    A = fc[:, :, 0, 0]
    Bm = fc[:, :, 0, 1]
    C = fc[:, :, 1, 0]
    Dm = fc[:, :, 1, 1]
    t1 = np.concatenate([A, Dm], axis=1).astype(np.float32)  # [S, HD]
    t2 = np.concatenate([Bm, C], axis=1).astype(np.float32)
    return np.ascontiguousarray(t1), np.ascontiguousarray(t2)


def _prep_masks(chunk=512):
    kpc = chunk // 128
    masks = np.zeros((kpc, 128, chunk), dtype=np.float32)
    q = np.arange(chunk)[None, :]
    p = np.arange(128)[:, None]
    for j in range(kpc):
        masks[j] = np.where(q >= j * 128 + p, 0.0, NEG).astype(np.float32)
    return masks


def _perm_head_rows(w):
    """Permute rows within each 128-row head block: evens first, odds second."""
    nh = w.shape[0] // HEAD_DIM
    return np.ascontiguousarray(
        w.reshape(nh, HEAD_DIM, -1)[:, _ROPE_PERM, :].reshape(w.shape)
    )


def _bf16(a):
    return np.ascontiguousarray(a.astype(ml_dtypes.bfloat16))


def _pmajor(a):
    """[T*128, F...] -> [128, T, F...] partition-major layout."""
    t = a.shape[0] // 128
    return np.ascontiguousarray(
        a.reshape(t, 128, *a.shape[1:]).swapaxes(0, 1)
    )


def make_core_inputs(x, freq_cis, wq, wk, wv, wo):
    """Build the 8 per-core input maps."""
    x = np.asarray(x, np.float32)
    wq = np.asarray(wq, np.float32)
    wk = np.asarray(wk, np.float32)
    wv = np.asarray(wv, np.float32)
    wo = np.asarray(wo, np.float32)
    t1, t2 = _prep_tables(freq_cis)
    masks = _prep_masks()
    ident = _bf16(np.eye(128, dtype=np.float32))
    IQ = HQ * HEAD_DIM

    in_maps = []
    for core in range(8):
        b, g = divmod(core, N_GROUPS)
        wq_g = _perm_head_rows(wq[g * IQ:(g + 1) * IQ])
        wk_g = _perm_head_rows(wk[g * HEAD_DIM:(g + 1) * HEAD_DIM])
        wv_g = wv[g * HEAD_DIM:(g + 1) * HEAD_DIM]
        wqT = _pmajor(_bf16(wq_g.T))
        wkvT = _pmajor(_bf16(np.concatenate([wk_g.T, wv_g.T], axis=1)))
        woT = _pmajor(_bf16(wo[:, g * IQ:(g + 1) * IQ].T))
        in_maps.append({
            "xT": _pmajor(_bf16(x[b].T)),
            "wqT": wqT,
            "wkvT": wkvT,
            "woT": woT,
            "t1": _pmajor(t1),
            "t2": _pmajor(t2),
            "masks": np.ascontiguousarray(masks.swapaxes(0, 1)),
            "ident": ident,
            "ones_col": _bf16(np.ones((128, 1), np.float32)),
            "ones_row": np.ones((1, 128), np.float32),
        })
    return in_maps


_CACHED_NC = None


def _get_nc():
    global _CACHED_NC
    if _CACHED_NC is None:
        from concourse import bacc

        nc = bacc.Bacc("TRN2", target_bir_lowering=False, debug=False)
        build_attention_core(nc)
        nc.compile()
        _CACHED_NC = nc
    return _CACHED_NC


def kernel(x, freq_cis, wq, wk, wv, wo):
    from concourse.bass_utils import run_bass_kernel_spmd

    nc = _get_nc()
    in_maps = make_core_inputs(x, freq_cis, wq, wk, wv, wo)
    res = run_bass_kernel_spmd(nc, in_maps, list(range(8)))
    out = np.zeros((B, S, DIM), dtype=np.float32)
    for core in range(8):
        b = core // N_GROUPS
        out[b] += res.results[core]["out_partial"]
    return out



# revision 10
# speedup vs baseline: 1.2213x; 1.2213x over previous
"""Trainium2 Bass kernel for GQA attention forward (B=2, S=2048, D=2048,
16 q-heads / 4 kv-heads, head_dim=128, RoPE, causal).

Sharding: 8 cores = 2 (batch) x 4 (kv-head groups). Each core computes its
batch's attention for one kv-head group (4 q-heads + 1 kv head) and a
row-parallel partial of the output projection; the host sums the 4 partials
per batch.

v2 layout (vs v1 baseline):
  - Phase A: single 768-wide matmul per (s-tile, d-tile) over merged
    [wq|wk|wv] weights (one LDWEIGHTS per d-tile instead of two).
  - Phase B: CHUNK=1024 q-chunks; per k-tile one QK matmul + one exp
    ACTIVATE (fewer, wider ScalarE ops — ScalarE was the phase bottleneck);
    causal diagonal handled by a post-exp 0/1 triangular multiply on
    GpSimd; softmax denominator accumulated on DVE into an f32r tile and
    broadcast via a single all-ones f32r matmul (frees the PE of the
    per-tile ones-column sum matmuls).
  - Phase C: 1024-wide output-projection chunks.
"""

import sys

if "/opt/trn_rl_repo" not in sys.path:
    sys.path.insert(0, "/opt/trn_rl_repo")

import numpy as np
import ml_dtypes

import concourse.bass as bass
import concourse.tile as tile
from concourse import mybir

F32 = mybir.dt.float32
F32R = mybir.dt.float32r
BF16 = mybir.dt.bfloat16

# Full-problem constants (per reference).
B, S, DIM = 2, 2048, 2048
N_HEADS, N_KV_HEADS, HEAD_DIM = 16, 4, 128
N_GROUPS = N_KV_HEADS          # tensor-parallel groups
HQ = N_HEADS // N_KV_HEADS     # q heads per group

# Diagonal-block causal mask: True = 0/1 multiply on GpSimd after exp,
# False = additive -1e30 mask on DVE before exp.
DIAG_GPSIMD = True


def build_attention_core(nc, S=S, D=DIM, HQ=HQ, HD=HEAD_DIM, CHUNK=1024):
    """Emit the per-core attention program into `nc` (Tile framework).

    Inputs (ExternalInput dram tensors):
      xT    [128, D/128, S] bf16   activations, transposed, partition-major
      wAllT [128, D/128, 768] bf16 [wq^T | wk^T | wv^T] (q/k RoPE-permuted)
      woT   [128, IQ/128, D] bf16  output projection slice, transposed
      t1,t2 [128, S/128, HD] f32   RoPE tables (permuted-half layout)
      tri   [128, 128] bf16        0/1 upper-triangular (incl diag) mask
      negtri [128, 128] f32        additive 0/-1e30 mask (fallback path)
      ones128 [128, 128] f32r      all-ones for rowsum+broadcast matmul
      ident [128, 128] bf16        identity for PE transposes
    Output:
      out_partial [S, D] f32
    """
    n_st = S // 128        # s tiles
    n_dt = D // 128        # d tiles
    n_ch = S // CHUNK      # q chunks (2)
    kpc = CHUNK // 128     # k-tiles per chunk (8)
    n_dc = D // CHUNK      # d chunks in phase C (2)
    IQ = HQ * HD           # 512
    PROJ = IQ + 2 * HD     # 768

    x_d = nc.dram_tensor("xT", [128, n_dt, S], BF16, kind="ExternalInput")
    wall_d = nc.dram_tensor("wAllT", [128, n_dt, PROJ], BF16,
                            kind="ExternalInput")
    woT_d = nc.dram_tensor("woT", [128, IQ // 128, D], BF16,
                           kind="ExternalInput")
    t1_d = nc.dram_tensor("t1", [128, n_st, HD], F32, kind="ExternalInput")
    t2_d = nc.dram_tensor("t2", [128, n_st, HD], F32, kind="ExternalInput")
    tri_d = nc.dram_tensor("tri", [128, 128], BF16, kind="ExternalInput")
    negtri_d = nc.dram_tensor("negtri", [128, 128], F32, kind="ExternalInput")
    ones_d = nc.dram_tensor("ones128", [128, 128], F32R, kind="ExternalInput")
    ident_d = nc.dram_tensor("ident", [128, 128], BF16, kind="ExternalInput")
    out_d = nc.dram_tensor("out_partial", [S, D], F32, kind="ExternalOutput")

    scale = float(HD) ** -0.5
    EXP = mybir.ActivationFunctionType.Exp

    with tile.TileContext(nc) as tc:
        with (
            tc.tile_pool(name="persist", bufs=1) as persist,
            tc.tile_pool(name="constB", bufs=1) as constB,
        ):
            qT_sb = persist.tile([128, HQ, S], BF16)    # [e, h, s]
            kT_sb = persist.tile([128, S], BF16)        # [e, s]
            v_sb = persist.tile([128, n_st, HD], BF16)  # [s_in_tile, s_tile, e]
            oT_sb = persist.tile([128, HQ, S], BF16)    # [e, h, s]

            # phase-B constants early (scalar queue) so nothing in B waits
            tri_sb = constB.tile([128, 128], BF16)
            nc.scalar.dma_start(out=tri_sb, in_=tri_d[:])
            negtri_sb = constB.tile([128, 128], F32)
            nc.scalar.dma_start(out=negtri_sb, in_=negtri_d[:])
            ones_sb = constB.tile([128, 128], F32R)
            nc.scalar.dma_start(out=ones_sb, in_=ones_d[:])

            # exp-table warm-up: a tiny Exp at t~0 hides the ~1.3us
            # ACT_TABLE_LOAD that would otherwise stall phase B's first exp
            warm_in = constB.tile([1, 1], F32)
            nc.vector.memset(warm_in, 0.0)
            warm_out = constB.tile([1, 1], F32)
            nc.scalar.activation(warm_out, warm_in, EXP)

            # ---------------- Phase A: projections + RoPE -------------------
            with (
                tc.tile_pool(name="weightsA", bufs=1) as weightsA,
                tc.tile_pool(name="xt", bufs=1) as xt_pool,
                tc.tile_pool(name="rope", bufs=4) as rope_pool,
                tc.tile_pool(name="ps_a", bufs=2, space="PSUM") as psa_pool,
                tc.tile_pool(name="ps_t", bufs=3, space="PSUM") as pst_pool,
            ):
                gq = n_dt // 4
                w_g = []
                for g in range(4):
                    wg = weightsA.tile([128, gq, PROJ], BF16, tag=f"w{g}",
                                       name=f"w{g}")
                    nc.scalar.dma_start(
                        out=wg, in_=wall_d[:, g * gq:(g + 1) * gq, :]
                    )
                    w_g.append(wg)
                ident = weightsA.tile([128, 128], BF16)
                nc.scalar.dma_start(out=ident, in_=ident_d[:])
                t1_sb = weightsA.tile([128, n_st, HD], F32)
                nc.scalar.dma_start(out=t1_sb, in_=t1_d[:])
                t2_sb = weightsA.tile([128, n_st, HD], F32)
                nc.scalar.dma_start(out=t2_sb, in_=t2_d[:])

                # pre-transposed activations: 2 d-tiles per DMA, sync queue
                xpair = []
                for g2 in range(n_dt // 2):
                    xt_t = xt_pool.tile([128, 2, S], BF16, tag=f"xt{g2}")
                    nc.sync.dma_start(out=xt_t, in_=x_d[:, g2 * 2:g2 * 2 + 2, :])
                    xpair.append(xt_t)
                xT = [xpair[d // 2][:, d % 2, :] for d in range(n_dt)]

                rope_pending = None

                def emit_transposes(rp):
                    q_rot_, k_rot_, sl_ = rp
                    for h in range(HQ):
                        ps_tq = pst_pool.tile([128, 128], BF16, tag="ps_t")
                        nc.tensor.transpose(
                            ps_tq, q_rot_[:, h * HD:(h + 1) * HD], ident
                        )
                        nc.scalar.copy(qT_sb[:, h, sl_], ps_tq)
                    ps_tk = pst_pool.tile([128, 128], BF16, tag="ps_t")
                    nc.tensor.transpose(ps_tk, k_rot_, ident)
                    nc.scalar.copy(kT_sb[:, sl_], ps_tk)

                for st in range(n_st):
                    ps_a = psa_pool.tile([128, PROJ], F32)
                    st_sl = slice(st * 128, (st + 1) * 128)
                    for dt_ in range(n_dt):
                        # matmul PSUM writes cannot cross a 2KB bank: split
                        # the 768-wide projection into 512 (q) + 256 (kv)
                        wslab = w_g[dt_ // gq][:, dt_ % gq, :]
                        nc.tensor.matmul(
                            ps_a[:, 0:IQ], xT[dt_][:, st_sl], wslab[:, 0:IQ],
                            start=(dt_ == 0), stop=(dt_ == n_dt - 1),
                        )
                        nc.tensor.matmul(
                            ps_a[:, IQ:PROJ], xT[dt_][:, st_sl],
                            wslab[:, IQ:PROJ],
                            start=(dt_ == 0), stop=(dt_ == n_dt - 1),
                        )
                    # previous s-tile's PE transposes: emitted here so PE
                    # never waits on the DVE RoPE chain
                    if rope_pending is not None:
                        emit_transposes(rope_pending)
                        rope_pending = None

                    # RoPE on all q heads at once (tables broadcast
                    # across heads via zero-stride AP)
                    t1s = t1_sb[:, st, :]
                    t2s = t2_sb[:, st, :]
                    t1b = bass.AP(tensor=t1s.tensor, offset=t1s.offset,
                                  ap=[t1s.ap[0], [0, HQ], t1s.ap[1]])
                    t2b = bass.AP(tensor=t2s.tensor, offset=t2s.offset,
                                  ap=[t2s.ap[0], [0, HQ], t2s.ap[1]])
                    ps_qv = ps_a[:, 0:IQ].rearrange("p (h e) -> p h e", h=HQ)
                    t1m = rope_pool.tile([128, HQ, HD], F32, tag="t1m")
                    nc.vector.tensor_mul(t1m, ps_qv, t1b)
                    t2m = rope_pool.tile([128, HQ, HD], F32, tag="t2m")
                    nc.vector.tensor_mul(
                        t2m[:, :, 0:64], ps_qv[:, :, 64:128], t2b[:, :, 0:64]
                    )
                    nc.vector.tensor_mul(
                        t2m[:, :, 64:128], ps_qv[:, :, 0:64], t2b[:, :, 64:128]
                    )
                    q_rot = rope_pool.tile([128, HQ * HD], BF16, tag="qrot")
                    nc.vector.tensor_add(
                        q_rot.rearrange("p (h e) -> p h e", h=HQ), t1m, t2m
                    )
                    # RoPE on k
                    ps_k = ps_a[:, IQ:IQ + HD]
                    t1mk = rope_pool.tile([128, HD], F32, tag="t1mk")
                    nc.vector.tensor_mul(t1mk, ps_k, t1_sb[:, st, :])
                    t2mk = rope_pool.tile([128, HD], F32, tag="t2mk")
                    nc.vector.tensor_mul(
                        t2mk[:, 0:64], ps_k[:, 64:128], t2_sb[:, st, 0:64]
                    )
                    nc.vector.tensor_mul(
                        t2mk[:, 64:128], ps_k[:, 0:64], t2_sb[:, st, 64:128]
                    )
                    k_rot = rope_pool.tile([128, HD], BF16, tag="krot")
                    nc.vector.tensor_add(k_rot, t1mk, t2mk)
                    rope_pending = (q_rot, k_rot, st_sl)

                    # v: straight copy (natural [s, e] layout), cast to bf16
                    nc.vector.tensor_copy(v_sb[:, st, :], ps_a[:, IQ + HD:PROJ])
                emit_transposes(rope_pending)

            # ---------------- Phases B+C shared: woT ------------------------
            with tc.tile_pool(name="weightsC", bufs=1) as weightsC:
                woT_sb = weightsC.tile([128, IQ // 128, D], BF16)
                nc.sync.dma_start(out=woT_sb, in_=woT_d[:])

                # ---------------- Phase B: attention ------------------------
                with (
                    tc.tile_pool(name="expt", bufs=4) as expt_pool,
                    tc.tile_pool(name="acc", bufs=2) as acc_pool,
                    tc.tile_pool(name="recip", bufs=2) as rec_pool,
                    tc.tile_pool(name="ps_s", bufs=2, space="PSUM") as pss_pool,
                    tc.tile_pool(name="ps_o", bufs=2, space="PSUM") as pso_pool,
                ):
                    norm_pending = [None]

                    def emit_norm():
                        ps_o_, acc_, h_, c_ = norm_pending[0]
                        norm_pending[0] = None
                        # f32r matmuls: all-ones [128,128] x acc -> every
                        # output partition holds the k-sum per q (rowsum and
                        # broadcast fused); split per 512-col PSUM bank
                        ps_nb = pss_pool.tile([128, CHUNK], F32, tag="ps_s")
                        for b in range(CHUNK // 512):
                            bs = slice(b * 512, (b + 1) * 512)
                            nc.tensor.matmul(ps_nb[:, bs], ones_sb,
                                             acc_[:, bs],
                                             start=True, stop=True)
                        recip = rec_pool.tile([128, CHUNK], F32, tag="recip")
                        nc.vector.reciprocal_approx_fast(recip, ps_nb)
                        nc.vector.tensor_mul(
                            oT_sb[:, h_, c_ * CHUNK:(c_ + 1) * CHUNK],
                            ps_o_, recip,
                        )

                    for c in range(n_ch):
                        for h in range(HQ):
                            ps_o = pso_pool.tile([128, CHUNK], F32, tag="ps_o")
                            acc = acc_pool.tile([128, CHUNK], F32R, tag="acc")
                            n_kj = (c + 1) * kpc
                            c_sl_base = c * CHUNK
                            pending = []

                            def flush_one():
                                kj_, off_, w_, et_ = pending.pop(0)
                                for b in range(CHUNK // 512):
                                    lo = max(off_, b * 512)
                                    hi = (b + 1) * 512
                                    if hi <= lo:
                                        continue
                                    nc.tensor.matmul(
                                        ps_o[:, lo:hi], v_sb[:, kj_, :],
                                        et_[:, lo:hi],
                                        start=(kj_ == 0),
                                        stop=(kj_ == n_kj - 1),
                                    )
                                with nc.allow_low_precision(
                                        reason="f32r denominator accumulator"):
                                    if kj_ == 0:
                                        nc.vector.tensor_copy(acc, et_)
                                    else:
                                        nc.vector.tensor_add(
                                            acc[:, off_:], acc[:, off_:],
                                            et_[:, off_:],
                                        )

                            for kj in range(n_kj):
                                off = max(0, kj * 128 - c_sl_base)
                                w = CHUNK - off
                                ps_s = pss_pool.tile([128, CHUNK], F32,
                                                     tag="ps_s")
                                for bq in range(CHUNK // 512):
                                    lo = max(off, bq * 512)
                                    hi = (bq + 1) * 512
                                    if hi <= lo:
                                        continue
                                    nc.tensor.matmul(
                                        ps_s[:, lo:hi],
                                        kT_sb[:, kj * 128:(kj + 1) * 128],
                                        qT_sb[:, h,
                                              c_sl_base + lo:c_sl_base + hi],
                                        start=True, stop=True,
                                    )
                                diag = kj * 128 >= c_sl_base
                                if diag and not DIAG_GPSIMD:
                                    nc.vector.tensor_add(
                                        ps_s[:, off:off + 128],
                                        ps_s[:, off:off + 128],
                                        negtri_sb,
                                    )
                                et = expt_pool.tile([128, CHUNK], BF16,
                                                    tag="expT")
                                nc.scalar.activation(
                                    et[:, off:CHUNK], ps_s[:, off:CHUNK],
                                    EXP, scale=scale
                                )
                                if diag and DIAG_GPSIMD:
                                    # zero the strictly-lower (k > q) part of
                                    # the 128x128 diagonal block, post-exp
                                    nc.gpsimd.tensor_mul(
                                        et[:, off:off + 128],
                                        et[:, off:off + 128], tri_sb
                                    )
                                pending.append((kj, off, w, et))
                                if kj == 1 and norm_pending[0] is not None:
                                    emit_norm()
                                if len(pending) > 2:
                                    flush_one()
                            while pending:
                                flush_one()
                            norm_pending[0] = (ps_o, acc, h, c)

                    emit_norm()

                # ---------------- Phase C: output projection ----------------
                with (
                    tc.tile_pool(name="outsb", bufs=2) as outsb_pool,
                    tc.tile_pool(name="ps_d", bufs=4, space="PSUM") as psd_pool,
                ):
                    DC = 512
                    for st in range(n_st):
                        out_sb = outsb_pool.tile([128, D], F32)
                        for dc in range(D // DC):
                            ps_d = psd_pool.tile([128, DC], F32)
                            for it in range(HQ):
                                nc.tensor.matmul(
                                    ps_d,
                                    oT_sb[:, it, st * 128:(st + 1) * 128],
                                    woT_sb[:, it, dc * DC:(dc + 1) * DC],
                                    start=(it == 0), stop=(it == HQ - 1),
                                )
                            nc.scalar.copy(
                                out_sb[:, dc * DC:(dc + 1) * DC], ps_d
                            )
                        nc.sync.dma_start(
                            out=out_d[st * 128:(st + 1) * 128, :], in_=out_sb
                        )

    return nc


# ---------------------------------------------------------------------------
# Host-side prep


_ROPE_PERM = np.concatenate([np.arange(0, HEAD_DIM, 2), np.arange(1, HEAD_DIM, 2)])


def _prep_tables(freq_cis, S_=S, HD_=HEAD_DIM):
    """RoPE tables in permuted-half layout: rot = q*t1 + swap(q)*t2."""
    fc = np.asarray(freq_cis, dtype=np.float32)
    A = fc[:, :, 0, 0]
    Bm = fc[:, :, 0, 1]
    C = fc[:, :, 1, 0]
    Dm = fc[:, :, 1, 1]
    t1 = np.concatenate([A, Dm], axis=1).astype(np.float32)  # [S, HD]
    t2 = np.concatenate([Bm, C], axis=1).astype(np.float32)
    return np.ascontiguousarray(t1), np.ascontiguousarray(t2)


def _perm_head_rows(w):
    """Permute rows within each 128-row head block: evens first, odds second."""
    nh = w.shape[0] // HEAD_DIM
    return np.ascontiguousarray(
        w.reshape(nh, HEAD_DIM, -1)[:, _ROPE_PERM, :].reshape(w.shape)
    )


def _bf16(a):
    return np.ascontiguousarray(a.astype(ml_dtypes.bfloat16))


def _pmajor(a):
    """[T*128, F...] -> [128, T, F...] partition-major layout."""
    t = a.shape[0] // 128
    return np.ascontiguousarray(
        a.reshape(t, 128, *a.shape[1:]).swapaxes(0, 1)
    )


def make_core_inputs(x, freq_cis, wq, wk, wv, wo):
    """Build the 8 per-core input maps."""
    x = np.asarray(x, np.float32)
    wq = np.asarray(wq, np.float32)
    wk = np.asarray(wk, np.float32)
    wv = np.asarray(wv, np.float32)
    wo = np.asarray(wo, np.float32)
    t1, t2 = _prep_tables(freq_cis)
    ident = _bf16(np.eye(128, dtype=np.float32))
    IQ = HQ * HEAD_DIM

    p = np.arange(128)[:, None]
    j = np.arange(128)[None, :]
    tri = _bf16((j >= p).astype(np.float32))           # 1 where q >= k
    negtri = np.where(j >= p, 0.0, -1e30).astype(np.float32)
    ones128 = np.ones((128, 128), np.float32)

    in_maps = []
    for core in range(8):
        b, g = divmod(core, N_GROUPS)
        wq_g = _perm_head_rows(wq[g * IQ:(g + 1) * IQ])
        wk_g = _perm_head_rows(wk[g * HEAD_DIM:(g + 1) * HEAD_DIM])
        wv_g = wv[g * HEAD_DIM:(g + 1) * HEAD_DIM]
        wall = np.concatenate([wq_g.T, wk_g.T, wv_g.T], axis=1)  # [D, 768]
        in_maps.append({
            "xT": _pmajor(_bf16(x[b].T)),
            "wAllT": _pmajor(_bf16(wall)),
            "woT": _pmajor(_bf16(wo[:, g * IQ:(g + 1) * IQ].T)),
            "t1": _pmajor(t1),
            "t2": _pmajor(t2),
            "tri": tri,
            "negtri": negtri,
            "ones128": ones128,
            "ident": ident,
        })
    return in_maps


_CACHED_NC = None


def _get_nc():
    global _CACHED_NC
    if _CACHED_NC is None:
        from concourse import bacc

        nc = bacc.Bacc("TRN2", target_bir_lowering=False, debug=False)
        build_attention_core(nc)
        nc.compile()
        _CACHED_NC = nc
    return _CACHED_NC


def kernel(x, freq_cis, wq, wk, wv, wo):
    from concourse.bass_utils import run_bass_kernel_spmd

    nc = _get_nc()
    in_maps = make_core_inputs(x, freq_cis, wq, wk, wv, wo)
    res = run_bass_kernel_spmd(nc, in_maps, list(range(8)))
    out = np.zeros((B, S, DIM), dtype=np.float32)
    for core in range(8):
        b = core // N_GROUPS
        out[b] += res.results[core]["out_partial"]
    return out


# revision 13
# speedup vs baseline: 1.2350x; 1.0112x over previous
"""Trainium2 Bass kernel for GQA attention forward (B=2, S=2048, D=2048,
16 q-heads / 4 kv-heads, head_dim=128, RoPE, causal).

Sharding: 8 cores = 2 (batch) x 4 (kv-head groups). Each core computes its
batch's attention for one kv-head group (4 q-heads + 1 kv head) and a
row-parallel partial of the output projection; the host sums the 4 partials
per batch.

v3 structure:
  - Phase A: merged [wq|wk|wv] weights, one LDWEIGHTS + 512/256 matmul pair
    per (s-tile, d-tile); x DMA'd in s-major chunks so the PE ramps as soon
    as the first 1MB lands instead of waiting for all 8MB.
  - Phase B: CHUNK=1024 q-chunks, one exp ACTIVATE per k-tile (ScalarE is
    the phase bottleneck; wide activations amortize its ~260ns/instr
    overhead); causal diagonal via post-exp 0/1 triangle multiply on GpSimd;
    softmax denominator accumulated on DVE (f32r) and rowsum+broadcast in a
    single all-ones f32r matmul per chunk.
  - Phase C: interleaved into phase B's second q-chunk (the PE idles ~25%
    there waiting on ScalarE exps); PSUM->SBUF copies ride DVE during the
    interleave and ScalarE in the tail; the two 512-col d-chunks of a pair
    share each LDWEIGHTS.
"""

import sys

if "/opt/trn_rl_repo" not in sys.path:
    sys.path.insert(0, "/opt/trn_rl_repo")

import numpy as np
import ml_dtypes

import concourse.bass as bass
import concourse.tile as tile
from concourse import mybir

F32 = mybir.dt.float32
F32R = mybir.dt.float32r
BF16 = mybir.dt.bfloat16

# Full-problem constants (per reference).
B, S, DIM = 2, 2048, 2048
N_HEADS, N_KV_HEADS, HEAD_DIM = 16, 4, 128
N_GROUPS = N_KV_HEADS          # tensor-parallel groups
HQ = N_HEADS // N_KV_HEADS     # q heads per group

# Diagonal-block causal mask: True = 0/1 multiply on GpSimd after exp,
# False = additive -1e30 mask on DVE before exp.
DIAG_GPSIMD = True


def build_attention_core(nc, S=S, D=DIM, HQ=HQ, HD=HEAD_DIM, CHUNK=1024):
    """Emit the per-core attention program into `nc` (Tile framework)."""
    n_st = S // 128        # s tiles
    n_dt = D // 128        # d tiles
    n_sc = 4               # x DMA s-chunks
    scw = S // n_sc        # 512 columns per s-chunk
    n_ch = S // CHUNK      # q chunks (2)
    kpc = CHUNK // 128     # k-tiles per chunk (8)
    IQ = HQ * HD           # 512
    PROJ = IQ + 2 * HD     # 768
    DC = 512               # phase C psum chunk width
    n_dc = D // DC

    x_d = nc.dram_tensor("xT", [128, n_sc, n_dt, scw], BF16,
                         kind="ExternalInput")
    wall_d = nc.dram_tensor("wAllT", [128, n_dt, PROJ], BF16,
                            kind="ExternalInput")
    woT_d = nc.dram_tensor("woT", [128, IQ // 128, D], BF16,
                           kind="ExternalInput")
    t1_d = nc.dram_tensor("t1", [128, n_st, HD], F32, kind="ExternalInput")
    t2_d = nc.dram_tensor("t2", [128, n_st, HD], F32, kind="ExternalInput")
    tri_d = nc.dram_tensor("tri", [128, 128], BF16, kind="ExternalInput")
    negtri_d = nc.dram_tensor("negtri", [128, 128], F32, kind="ExternalInput")
    ones_d = nc.dram_tensor("ones128", [128, 128], F32R, kind="ExternalInput")
    ident_d = nc.dram_tensor("ident", [128, 128], BF16, kind="ExternalInput")
    out_d = nc.dram_tensor("out_partial", [S, D], F32, kind="ExternalOutput")

    scale = float(HD) ** -0.5
    EXP = mybir.ActivationFunctionType.Exp

    with tile.TileContext(nc) as tc:
        with (
            tc.tile_pool(name="persist", bufs=1) as persist,
            tc.tile_pool(name="constB", bufs=1) as constB,
        ):
            qT_sb = persist.tile([128, HQ, S], BF16)    # [e, h, s]
            kT_sb = persist.tile([128, S], BF16)        # [e, s]
            v_sb = persist.tile([128, n_st, HD], BF16)  # [s_in_tile, s_tile, e]
            oT_sb = persist.tile([128, HQ, S], BF16)    # [e, h, s]

            # phase-B constants early (scalar queue) so nothing in B waits
            tri_sb = constB.tile([128, 128], BF16)
            nc.scalar.dma_start(out=tri_sb, in_=tri_d[:])
            negtri_sb = constB.tile([128, 128], F32)
            nc.scalar.dma_start(out=negtri_sb, in_=negtri_d[:])
            ones_sb = constB.tile([128, 128], F32R)
            nc.scalar.dma_start(out=ones_sb, in_=ones_d[:])

            # exp-table warm-up: a tiny Exp at t~0 hides the ~1.3us
            # ACT_TABLE_LOAD that would otherwise stall phase B's first exp
            warm_in = constB.tile([1, 1], F32)
            nc.vector.memset(warm_in, 0.0)
            warm_out = constB.tile([1, 1], F32)
            nc.scalar.activation(warm_out, warm_in, EXP)

            # ---------------- Phase A: projections + RoPE -------------------
            with (
                tc.tile_pool(name="weightsA", bufs=1) as weightsA,
                tc.tile_pool(name="xt", bufs=1) as xt_pool,
                tc.tile_pool(name="rope", bufs=4) as rope_pool,
                tc.tile_pool(name="ps_a", bufs=2, space="PSUM") as psa_pool,
                tc.tile_pool(name="ps_t", bufs=3, space="PSUM") as pst_pool,
            ):
                gq = n_dt // 4
                w_g = []
                for g in range(4):
                    wg = weightsA.tile([128, gq, PROJ], BF16, tag=f"w{g}",
                                       name=f"w{g}")
                    nc.scalar.dma_start(
                        out=wg, in_=wall_d[:, g * gq:(g + 1) * gq, :]
                    )
                    w_g.append(wg)
                ident = weightsA.tile([128, 128], BF16)
                nc.scalar.dma_start(out=ident, in_=ident_d[:])
                t1_sb = weightsA.tile([128, n_st, HD], F32)
                nc.scalar.dma_start(out=t1_sb, in_=t1_d[:])
                t2_sb = weightsA.tile([128, n_st, HD], F32)
                nc.scalar.dma_start(out=t2_sb, in_=t2_d[:])

                # x in s-major chunks: all 16 d-tiles of a 512-col s-range
                # land per DMA, so s-tile processing starts after ~1MB
                xsc = []
                for sc in range(n_sc):
                    xt_t = xt_pool.tile([128, n_dt, scw], BF16, tag=f"x{sc}")
                    nc.sync.dma_start(out=xt_t, in_=x_d[:, sc, :, :])
                    xsc.append(xt_t)

                rope_pending = None

                def emit_transposes(rp):
                    q_rot_, k_rot_, sl_ = rp
                    for h in range(HQ):
                        ps_tq = pst_pool.tile([128, 128], BF16, tag="ps_t")
                        nc.tensor.transpose(
                            ps_tq, q_rot_[:, h * HD:(h + 1) * HD], ident
                        )
                        nc.scalar.copy(qT_sb[:, h, sl_], ps_tq)
                    ps_tk = pst_pool.tile([128, 128], BF16, tag="ps_t")
                    nc.tensor.transpose(ps_tk, k_rot_, ident)
                    nc.scalar.copy(kT_sb[:, sl_], ps_tk)

                for st in range(n_st):
                    ps_a = psa_pool.tile([128, PROJ], F32)
                    st_sl = slice(st * 128, (st + 1) * 128)
                    xloc = slice((st % 4) * 128, (st % 4) * 128 + 128)
                    for dt_ in range(n_dt):
                        xop = xsc[st // 4][:, dt_, xloc]
                        wslab = w_g[dt_ // gq][:, dt_ % gq, :]
                        # matmul PSUM writes cannot cross a 2KB bank: split
                        # 768 into 512 (q) + 256 (kv); same stationary
                        nc.tensor.matmul(
                            ps_a[:, 0:IQ], xop, wslab[:, 0:IQ],
                            start=(dt_ == 0), stop=(dt_ == n_dt - 1),
                        )
                        nc.tensor.matmul(
                            ps_a[:, IQ:PROJ], xop, wslab[:, IQ:PROJ],
                            start=(dt_ == 0), stop=(dt_ == n_dt - 1),
                        )
                    # previous s-tile's PE transposes: emitted here so PE
                    # never waits on the DVE RoPE chain
                    if rope_pending is not None:
                        emit_transposes(rope_pending)
                        rope_pending = None

                    # RoPE on all q heads at once (tables broadcast
                    # across heads via zero-stride AP)
                    t1s = t1_sb[:, st, :]
                    t2s = t2_sb[:, st, :]
                    t1b = bass.AP(tensor=t1s.tensor, offset=t1s.offset,
                                  ap=[t1s.ap[0], [0, HQ], t1s.ap[1]])
                    t2b = bass.AP(tensor=t2s.tensor, offset=t2s.offset,
                                  ap=[t2s.ap[0], [0, HQ], t2s.ap[1]])
                    ps_qv = ps_a[:, 0:IQ].rearrange("p (h e) -> p h e", h=HQ)
                    t1m = rope_pool.tile([128, HQ, HD], F32, tag="t1m")
                    nc.vector.tensor_mul(t1m, ps_qv, t1b)
                    t2m = rope_pool.tile([128, HQ, HD], F32, tag="t2m")
                    nc.vector.tensor_mul(
                        t2m[:, :, 0:64], ps_qv[:, :, 64:128], t2b[:, :, 0:64]
                    )
                    nc.vector.tensor_mul(
                        t2m[:, :, 64:128], ps_qv[:, :, 0:64], t2b[:, :, 64:128]
                    )
                    q_rot = rope_pool.tile([128, HQ * HD], BF16, tag="qrot")
                    nc.vector.tensor_add(
                        q_rot.rearrange("p (h e) -> p h e", h=HQ), t1m, t2m
                    )
                    # RoPE on k
                    ps_k = ps_a[:, IQ:IQ + HD]
                    t1mk = rope_pool.tile([128, HD], F32, tag="t1mk")
                    nc.vector.tensor_mul(t1mk, ps_k, t1_sb[:, st, :])
                    t2mk = rope_pool.tile([128, HD], F32, tag="t2mk")
                    nc.vector.tensor_mul(
                        t2mk[:, 0:64], ps_k[:, 64:128], t2_sb[:, st, 0:64]
                    )
                    nc.vector.tensor_mul(
                        t2mk[:, 64:128], ps_k[:, 0:64], t2_sb[:, st, 64:128]
                    )
                    k_rot = rope_pool.tile([128, HD], BF16, tag="krot")
                    nc.vector.tensor_add(k_rot, t1mk, t2mk)
                    rope_pending = (q_rot, k_rot, st_sl)

                    # v: straight copy (natural [s, e] layout), cast to bf16
                    nc.vector.tensor_copy(v_sb[:, st, :], ps_a[:, IQ + HD:PROJ])
                emit_transposes(rope_pending)

            # ---------------- Phases B+C ------------------------------------
            with (
                tc.tile_pool(name="weightsC", bufs=1) as weightsC,
                tc.tile_pool(name="expt", bufs=4) as expt_pool,
                tc.tile_pool(name="acc", bufs=2) as acc_pool,
                tc.tile_pool(name="recip", bufs=2) as rec_pool,
                tc.tile_pool(name="outsb", bufs=2) as outsb_pool,
                tc.tile_pool(name="ps_s", bufs=2, space="PSUM") as pss_pool,
                tc.tile_pool(name="ps_o", bufs=1, space="PSUM") as pso_pool,
                tc.tile_pool(name="ps_d", bufs=1, space="PSUM") as psd_pool,
            ):
                woT_sb = weightsC.tile([128, IQ // 128, D], BF16)
                nc.sync.dma_start(out=woT_sb, in_=woT_d[:])

                # ---- phase C emission helpers (st-tile = 4 dc-pair groups)
                out_sb_by_st = {}

                def emit_c_group(st, pair, on_dve):
                    if pair == 0:
                        out_sb_by_st[st] = outsb_pool.tile(
                            [128, D], F32, tag="outsb", name=f"outsb{st}"
                        )
                    out_sb = out_sb_by_st[st]
                    st_sl = slice(st * 128, (st + 1) * 128)
                    pd = [psd_pool.tile([128, DC], F32, tag=f"ps_d{i}",
                                        name=f"ps_d{st}_{pair}_{i}")
                          for i in range(2)]
                    # the two d-chunks of a pair share each oT LDWEIGHTS
                    for it in range(HQ):
                        for i in range(2):
                            dc = 2 * pair + i
                            nc.tensor.matmul(
                                pd[i], oT_sb[:, it, st_sl],
                                woT_sb[:, it, dc * DC:(dc + 1) * DC],
                                start=(it == 0), stop=(it == HQ - 1),
                            )
                    for i in range(2):
                        dc = 2 * pair + i
                        dst = out_sb[:, dc * DC:(dc + 1) * DC]
                        if on_dve:
                            nc.vector.tensor_copy(dst, pd[i])
                        else:
                            nc.scalar.copy(dst, pd[i])
                    if pair == n_dc // 2 - 1:
                        nc.sync.dma_start(out=out_d[st_sl, :], in_=out_sb)
                        del out_sb_by_st[st]

                c_jobs = [(st, pair) for st in range(n_st)
                          for pair in range(n_dc // 2)]

                # ---------------- Phase B: attention ------------------------
                norm_pending = [None]

                def emit_norm():
                    ps_o_, acc_, h_, c_ = norm_pending[0]
                    norm_pending[0] = None
                    # f32r matmuls: all-ones [128,128] x acc -> every output
                    # partition holds the k-sum per q (rowsum + broadcast
                    # fused); split per 512-col PSUM bank
                    ps_nb = pss_pool.tile([128, CHUNK], F32, tag="ps_s")
                    for b in range(CHUNK // 512):
                        bs = slice(b * 512, (b + 1) * 512)
                        nc.tensor.matmul(ps_nb[:, bs], ones_sb, acc_[:, bs],
                                         start=True, stop=True)
                    recip = rec_pool.tile([128, CHUNK], F32, tag="recip")
                    nc.vector.reciprocal_approx_fast(recip, ps_nb)
                    nc.vector.tensor_mul(
                        oT_sb[:, h_, c_ * CHUNK:(c_ + 1) * CHUNK],
                        ps_o_, recip,
                    )

                steps = [0]

                for c in range(n_ch):
                    for h in range(HQ):
                        ps_o = pso_pool.tile([128, CHUNK], F32, tag="ps_o")
                        acc = acc_pool.tile([128, CHUNK], F32R, tag="acc")
                        n_kj = (c + 1) * kpc
                        c_sl_base = c * CHUNK
                        pending = []

                        def flush_one():
                            kj_, off_, et_ = pending.pop(0)
                            for b in range(CHUNK // 512):
                                lo = max(off_, b * 512)
                                hi = (b + 1) * 512
                                if hi <= lo:
                                    continue
                                nc.tensor.matmul(
                                    ps_o[:, lo:hi], v_sb[:, kj_, :],
                                    et_[:, lo:hi],
                                    start=(kj_ == 0), stop=(kj_ == n_kj - 1),
                                )
                            with nc.allow_low_precision(
                                    reason="f32r denominator accumulator"):
                                if kj_ == 0:
                                    nc.vector.tensor_copy(acc, et_)
                                else:
                                    nc.vector.tensor_add(
                                        acc[:, off_:], acc[:, off_:],
                                        et_[:, off_:],
                                    )

                        for kj in range(n_kj):
                            off = max(0, kj * 128 - c_sl_base)
                            ps_s = pss_pool.tile([128, CHUNK], F32,
                                                 tag="ps_s")
                            for bq in range(CHUNK // 512):
                                lo = max(off, bq * 512)
                                hi = (bq + 1) * 512
                                if hi <= lo:
                                    continue
                                nc.tensor.matmul(
                                    ps_s[:, lo:hi],
                                    kT_sb[:, kj * 128:(kj + 1) * 128],
                                    qT_sb[:, h,
                                          c_sl_base + lo:c_sl_base + hi],
                                    start=True, stop=True,
                                )
                            diag = kj * 128 >= c_sl_base
                            if diag and not DIAG_GPSIMD:
                                nc.vector.tensor_add(
                                    ps_s[:, off:off + 128],
                                    ps_s[:, off:off + 128], negtri_sb,
                                )
                            et = expt_pool.tile([128, CHUNK], BF16,
                                                tag="expT")
                            nc.scalar.activation(
                                et[:, off:CHUNK], ps_s[:, off:CHUNK],
                                EXP, scale=scale
                            )
                            if diag and DIAG_GPSIMD:
                                # zero the strictly-lower (k > q) part of the
                                # 128x128 diagonal block, post-exp
                                nc.gpsimd.tensor_mul(
                                    et[:, off:off + 128],
                                    et[:, off:off + 128], tri_sb
                                )
                            pending.append((kj, off, et))
                            if kj == 1 and norm_pending[0] is not None:
                                emit_norm()
                            if len(pending) > 2:
                                flush_one()
                            # interleave phase C into the second q-chunk:
                            # the PE has slack here (exp-rate bound)
                            if c == n_ch - 1:
                                steps[0] += 1
                                if steps[0] % 4 == 0 and steps[0] >= 4 and \
                                        c_jobs and c_jobs[0][0] < 8:
                                    emit_c_group(*c_jobs.pop(0), on_dve=True)
                        while pending:
                            flush_one()
                        norm_pending[0] = (ps_o, acc, h, c)

                emit_norm()

                # ---------------- Phase C tail ------------------------------
                while c_jobs:
                    emit_c_group(*c_jobs.pop(0), on_dve=False)

    return nc


# ---------------------------------------------------------------------------
# Host-side prep


_ROPE_PERM = np.concatenate([np.arange(0, HEAD_DIM, 2), np.arange(1, HEAD_DIM, 2)])


def _prep_tables(freq_cis, S_=S, HD_=HEAD_DIM):
    """RoPE tables in permuted-half layout: rot = q*t1 + swap(q)*t2."""
    fc = np.asarray(freq_cis, dtype=np.float32)
    A = fc[:, :, 0, 0]
    Bm = fc[:, :, 0, 1]
    C = fc[:, :, 1, 0]
    Dm = fc[:, :, 1, 1]
    t1 = np.concatenate([A, Dm], axis=1).astype(np.float32)  # [S, HD]
    t2 = np.concatenate([Bm, C], axis=1).astype(np.float32)
    return np.ascontiguousarray(t1), np.ascontiguousarray(t2)


def _perm_head_rows(w):
    """Permute rows within each 128-row head block: evens first, odds second."""
    nh = w.shape[0] // HEAD_DIM
    return np.ascontiguousarray(
        w.reshape(nh, HEAD_DIM, -1)[:, _ROPE_PERM, :].reshape(w.shape)
    )


def _bf16(a):
    return np.ascontiguousarray(a.astype(ml_dtypes.bfloat16))


def _pmajor(a):
    """[T*128, F...] -> [128, T, F...] partition-major layout."""
    t = a.shape[0] // 128
    return np.ascontiguousarray(
        a.reshape(t, 128, *a.shape[1:]).swapaxes(0, 1)
    )


def make_core_inputs(x, freq_cis, wq, wk, wv, wo):
    """Build the 8 per-core input maps."""
    x = np.asarray(x, np.float32)
    wq = np.asarray(wq, np.float32)
    wk = np.asarray(wk, np.float32)
    wv = np.asarray(wv, np.float32)
    wo = np.asarray(wo, np.float32)
    t1, t2 = _prep_tables(freq_cis)
    ident = _bf16(np.eye(128, dtype=np.float32))
    IQ = HQ * HEAD_DIM

    p = np.arange(128)[:, None]
    j = np.arange(128)[None, :]
    tri = _bf16((j >= p).astype(np.float32))           # 1 where q >= k
    negtri = np.where(j >= p, 0.0, -1e30).astype(np.float32)
    ones128 = np.ones((128, 128), np.float32)

    in_maps = []
    for core in range(8):
        b, g = divmod(core, N_GROUPS)
        wq_g = _perm_head_rows(wq[g * IQ:(g + 1) * IQ])
        wk_g = _perm_head_rows(wk[g * HEAD_DIM:(g + 1) * HEAD_DIM])
        wv_g = wv[g * HEAD_DIM:(g + 1) * HEAD_DIM]
        wall = np.concatenate([wq_g.T, wk_g.T, wv_g.T], axis=1)  # [D, 768]
        # x: [128, 16, 2048] partition-major -> s-chunk-major
        # [128, 4(sc), 16(dt), 512]
        xp = _pmajor(_bf16(x[b].T))
        xp = np.ascontiguousarray(
            xp.reshape(128, 16, 4, 512).transpose(0, 2, 1, 3)
        )
        in_maps.append({
            "xT": xp,
            "wAllT": _pmajor(_bf16(wall)),
            "woT": _pmajor(_bf16(wo[:, g * IQ:(g + 1) * IQ].T)),
            "t1": _pmajor(t1),
            "t2": _pmajor(t2),
            "tri": tri,
            "negtri": negtri,
            "ones128": ones128,
            "ident": ident,
        })
    return in_maps


_CACHED_NC = None


def _get_nc():
    global _CACHED_NC
    if _CACHED_NC is None:
        from concourse import bacc

        nc = bacc.Bacc("TRN2", target_bir_lowering=False, debug=False)
        build_attention_core(nc)
        nc.compile()
        _CACHED_NC = nc
    return _CACHED_NC


def kernel(x, freq_cis, wq, wk, wv, wo):
    from concourse.bass_utils import run_bass_kernel_spmd

    nc = _get_nc()
    in_maps = make_core_inputs(x, freq_cis, wq, wk, wv, wo)
    res = run_bass_kernel_spmd(nc, in_maps, list(range(8)))
    out = np.zeros((B, S, DIM), dtype=np.float32)
    for core in range(8):
        b = core // N_GROUPS
        out[b] += res.results[core]["out_partial"]
    return out
